# revision 1
# baseline (speedup 1.0000x reference)
"""BindingPocketGNN (3-layer GCN, N=50000, E=800000) on 8 Trainium2 NeuronCores.

Distribution: nodes sharded into 8 contiguous ranges (6250/core). Each core owns the
scatter/aggregation for its destination-node range; edges are routed (host-side) to the
core owning their destination. Source features are gathered from a replicated
node-major table (input x for layer 1; AllGather-replicated activations for layers 2/3).

Per layer, on each core (feat-major formulation so BN/bias are per-partition):
    z^T[f_in, d]  = sum_e  msg_e[f_in] * mask[e, d]      (TensorE: msg.T @ (iota==dst) mask)
    zs            = z^T * dinv[dst]                      (fused in PSUM->SBUF copy)
    y^T[f_out, d] = W.T @ zs                             (TensorE)
    stats         = AllReduce(sum/sumsq of y)            (1KB collective; BN layers)
    act^T         = Relu(A*y^T + B)                      (ScalarE, per-partition A/B)
    h             = act^T transposed to node-major       (TensorE transpose)
    table_{l+1}   = AllGather(h * ...)                   (collective; layers 1,2)
Layer 3 feeds a [128->1] FC matmul; +fcb and sigmoid applied on host.

deg/dinv and dinv[src] pre-scaling of x are computed on host (pure input transforms);
per-edge norm = dinv[src]*dinv[dst] is realized as table-prescale x dinv_bcast.
"""
import sys
if "/opt/trn_rl_repo" not in sys.path:
    sys.path.insert(0, "/opt/trn_rl_repo")

import numpy as np

import concourse.bass as bass
import concourse.bacc as bacc
import concourse.mybir as mybir
import concourse.tile as tile
from concourse import bass_utils
from concourse.masks import make_identity

N = 50000
E = 800000
IN, HID = 64, 128
BN_EPS = 1e-5
NCORES = 8
NPC = N // NCORES          # 6250 nodes per core
P = 128
NT = (NPC + P - 1) // P    # 49 dst tiles per core
LAST_D = NPC - (NT - 1) * P  # 106

BF16_TABLES = True         # gather tables + masks in bf16 (fp32 accumulation in PSUM)

F32 = mybir.dt.float32
I32 = mybir.dt.int32
BF16 = mybir.dt.bfloat16
DT_TAB = BF16 if BF16_TABLES else F32
NP_TAB = np.dtype("bfloat16") if False else None  # numpy bf16 via ml_dtypes below

import ml_dtypes
NP_TAB = np.dtype(ml_dtypes.bfloat16) if BF16_TABLES else np.dtype(np.float32)
import os
STAGE = int(os.environ.get("GCN_STAGE", "6"))
REPS = int(os.environ.get("GCN_REPS", "1"))
NOCOLL = os.environ.get("GCN_NOCOLL", "0") == "1"
AGLOCAL = os.environ.get("GCN_AGLOCAL", "0") == "1"
SFRAC = float(os.environ.get("GCN_SFRAC", "1.0"))
SKIPW = os.environ.get("GCN_SKIPW", "0") == "1"

Alu = mybir.AluOpType
Act = mybir.ActivationFunctionType

_NC_CACHE = {}


def _build(T, S):
    """Build+schedule the SPMD program. T = total edge subtiles, S = list of subtile
    counts per dst tile (len NT, sum T). Identical for all 8 cores."""
    nc = bacc.Bacc("TRN2", target_bir_lowering=False, debug=False, num_devices=NCORES)

    # ---- I/O ----
    xs = nc.dram_tensor("xs", [N, IN], DT_TAB, kind="ExternalInput")
    gidx_d = nc.dram_tensor("gidx", [P, T], I32, kind="ExternalInput")
    dloc_d = nc.dram_tensor("dloc", [P, T], F32, kind="ExternalInput")
    dinv_d = nc.dram_tensor("dinv_sl", [P, NT], F32, kind="ExternalInput")
    W_d = [
        nc.dram_tensor("W1", [IN, HID], F32, kind="ExternalInput"),
        nc.dram_tensor("W2", [HID, HID], F32, kind="ExternalInput"),
        nc.dram_tensor("W3", [HID, HID], F32, kind="ExternalInput"),
    ]
    fcW_d = nc.dram_tensor("fcW", [HID, 1], F32, kind="ExternalInput")
    g_d = [nc.dram_tensor("g1", [HID, 1], F32, kind="ExternalInput"),
           nc.dram_tensor("g2", [HID, 1], F32, kind="ExternalInput")]
    bt_d = [nc.dram_tensor("bt1", [HID, 1], F32, kind="ExternalInput"),
            nc.dram_tensor("bt2", [HID, 1], F32, kind="ExternalInput")]
    b3_d = nc.dram_tensor("b3", [HID, 1], F32, kind="ExternalInput")
    outv = nc.dram_tensor("outv", [1, NPC], F32, kind="ExternalOutput")

    with tile.TileContext(nc) as tc:
        with (
            tc.tile_pool(name="meta", bufs=1) as meta,
            tc.tile_pool(name="msgp", bufs=12) as msgp,
            tc.tile_pool(name="maskp", bufs=12) as maskp,
            tc.tile_pool(name="zsp", bufs=3) as zsp,
            tc.tile_pool(name="actp", bufs=3) as actp,
            tc.tile_pool(name="hp", bufs=3) as hp,
            tc.tile_pool(name="sqp", bufs=2) as sqp,
            tc.tile_pool(name="zps_p", bufs=2, space="PSUM") as zps_p,
            tc.tile_pool(name="yps_p", bufs=2, space="PSUM") as yps_p,
            tc.tile_pool(name="trps_p", bufs=2, space="PSUM") as trps_p,
            tc.tile_pool(name="fcps_p", bufs=1, space="PSUM") as fcps_p,
            tc.tile_pool(name="dram", bufs=1, space="DRAM") as dram,
        ):
            # ---- resident metadata ----
            gidx_sb = meta.tile([P, T], I32)
            nc.sync.dma_start(gidx_sb[:], gidx_d[:])
            dloc_sb = meta.tile([P, T], F32)
            nc.sync.dma_start(dloc_sb[:], dloc_d[:])
            dinv_sl = meta.tile([P, NT], F32)
            nc.sync.dma_start(dinv_sl[:], dinv_d[:])
            W_sb = []
            for l in range(3):
                fi = IN if l == 0 else HID
                w = meta.tile([fi, HID], F32, name=f"W{l}_sb")
                nc.sync.dma_start(w[:], W_d[l][:])
                W_sb.append(w)
            fcW_sb = meta.tile([HID, 1], F32)
            nc.sync.dma_start(fcW_sb[:], fcW_d[:])
            g_sb, bt_sb = [], []
            for l in range(2):
                gg = meta.tile([HID, 1], F32, name=f"g{l}_sb")
                nc.sync.dma_start(gg[:], g_d[l][:])
                g_sb.append(gg)
                bb = meta.tile([HID, 1], F32, name=f"bt{l}_sb")
                nc.sync.dma_start(bb[:], bt_d[l][:])
                bt_sb.append(bb)
            b3_sb = meta.tile([HID, 1], F32)
            nc.sync.dma_start(b3_sb[:], b3_d[:])
            eps_sb = meta.tile([P, 1], F32)
            nc.vector.memset(eps_sb[:], BN_EPS)

            ident = meta.tile([P, P], F32)
            make_identity(nc, ident[:])
            iota_i = meta.tile([P, P], I32)
            nc.gpsimd.iota(iota_i[:], pattern=[[1, P]], base=0, channel_multiplier=0)
            iota_t = meta.tile([P, P], DT_TAB)
            nc.vector.tensor_copy(iota_t[:], iota_i[:])

            # dinv broadcast rows: dinv_bc[:, t*128+j] = dinv of node t*128+j (all partitions)
            dinv_bc = meta.tile([P, NT * P], F32)
            for t in range(NT):
                tr = trps_p.tile([P, P], F32, tag="tr")
                nc.tensor.transpose(tr[:], dinv_sl[:, t:t + 1].to_broadcast([P, P]), ident[:])
                nc.vector.tensor_copy(dinv_bc[:, t * P:(t + 1) * P], tr[:])

            ystore = meta.tile([P, NT * P], F32)
            sums = meta.tile([P, NT], F32)
            sumsq = meta.tile([P, NT], F32)
            out_store = meta.tile([1, NPC], F32)

            # internal DRAM for collectives (fresh per rep: Shared tensors allow one writer)
            def mk_coll(rep):
                tab_in = [dram.tile([NPC, HID], DT_TAB, name=f"tab{l}_in_r{rep}") for l in (1, 2)]
                tab_out = [dram.tile([N, HID], DT_TAB, name=f"tab{l}_out_r{rep}",
                                     addr_space="Local" if (NOCOLL or AGLOCAL) else "Shared")
                           for l in (1, 2)]
                st_in = [dram.tile([P, 2], F32, name=f"st{l}_in_r{rep}") for l in (0, 1)]
                st_out = [dram.tile([P, 2], F32, name=f"st{l}_out_r{rep}", addr_space="Shared")
                          for l in (0, 1)]
                return tab_in, tab_out, st_in, st_out

            off = [0]
            for t in range(NT):
                off.append(off[-1] + S[t])

            n_layers = 1 if STAGE <= 3 else (2 if STAGE <= 5 else 3)
            for _rep in range(REPS):
              tab_in, tab_out, st_in, st_out = mk_coll(_rep)
              for l in range(n_layers):
                 f_in = IN if l == 0 else HID
                 table = xs if l == 0 else tab_out[l - 1]
                 # ---- aggregation + weight matmul ----
                 for t in range(NT):
                     d_hi = LAST_D if t == NT - 1 else P
                     zps = zps_p.tile([P, P], F32, tag="zps")
                     for s in range(S[t]):
                         g = off[t] + s
                         msg = msgp.tile([P, f_in], DT_TAB, tag="msg")
                         nc.gpsimd.indirect_dma_start(
                             out=msg[:], out_offset=None, in_=table[:],
                             in_offset=bass.IndirectOffsetOnAxis(ap=gidx_sb[:, g:g + 1], axis=0),
                         )
                         mask = maskp.tile([P, P], DT_TAB, tag="mask")
                         nc.vector.tensor_scalar(
                             out=mask[:], in0=iota_t[:], scalar1=dloc_sb[:, g:g + 1],
                             scalar2=None, op0=Alu.is_equal,
                         )
                         nc.tensor.matmul(zps[:f_in, :], lhsT=msg[:], rhs=mask[:],
                                          start=(s == 0), stop=(s == S[t] - 1))
                     zs = zsp.tile([P, P], F32, tag="zs")
                     nc.vector.tensor_tensor(
                         out=zs[:f_in, :], in0=zps[:f_in, :],
                         in1=dinv_bc[:f_in, t * P:(t + 1) * P], op=Alu.mult,
                     )
                     yps = yps_p.tile([P, P], F32, tag="yps")
                     nc.tensor.matmul(yps[:], lhsT=W_sb[l][:], rhs=zs[:f_in, :],
                                      start=True, stop=True)
                     if l < 2:
                         nc.scalar.activation(
                             out=ystore[:, t * P:t * P + d_hi], in_=yps[:, :d_hi],
                             func=Act.Copy, accum_out=sums[:, t:t + 1],
                         )
                         sq = sqp.tile([P, P], F32, tag="sq")
                         nc.scalar.activation(
                             out=sq[:, :d_hi], in_=yps[:, :d_hi],
                             func=Act.Square, accum_out=sumsq[:, t:t + 1],
                         )
                     else:
                         act3 = actp.tile([P, P], F32, tag="act")
                         nc.scalar.activation(out=act3[:, :d_hi], in_=yps[:, :d_hi],
                                              func=Act.Relu, bias=b3_sb[:], scale=1.0)
                         fcp = fcps_p.tile([1, P], F32, tag="fcp")
                         nc.tensor.matmul(fcp[:1, :d_hi], lhsT=fcW_sb[:], rhs=act3[:, :d_hi],
                                          start=True, stop=True)
                         nc.vector.tensor_copy(out_store[:1, t * P:t * P + d_hi], fcp[:1, :d_hi])

                 if STAGE == 1:
                     nc.vector.tensor_copy(out_store[:1, :P], ystore[:1, :P])
                     break
                 if l < 2:
                     # ---- BN stats allreduce + coefficients ----
                     stats = meta.tile([P, 2], F32, name=f"stats{l}")
                     nc.vector.tensor_reduce(stats[:, 0:1], sums[:], axis=mybir.AxisListType.X, op=Alu.add)
                     nc.vector.tensor_reduce(stats[:, 1:2], sumsq[:], axis=mybir.AxisListType.X, op=Alu.add)
                     nc.sync.dma_start(st_in[l][:], stats[:])
                     nc.gpsimd.collective_compute(
                         "AllReduce", Alu.add, replica_groups=[list(range(NCORES))],
                         ins=[st_in[l][:]], outs=[st_out[l][:]],
                     )
                     tot = meta.tile([P, 2], F32, name=f"tot{l}")
                     nc.sync.dma_start(tot[:], st_out[l][:])
                     cf = meta.tile([P, 6], F32, name=f"cf{l}")  # mean ex2 var std A B
                     nc.vector.tensor_scalar_mul(cf[:, 0:1], tot[:, 0:1], 1.0 / N)
                     nc.vector.tensor_scalar_mul(cf[:, 1:2], tot[:, 1:2], 1.0 / N)
                     nc.vector.tensor_tensor(out=cf[:, 2:3], in0=cf[:, 0:1], in1=cf[:, 0:1], op=Alu.mult)
                     nc.vector.tensor_tensor(out=cf[:, 2:3], in0=cf[:, 1:2], in1=cf[:, 2:3], op=Alu.subtract)
                     nc.scalar.activation(out=cf[:, 3:4], in_=cf[:, 2:3], func=Act.Sqrt, bias=eps_sb[:], scale=1.0)
                     nc.vector.reciprocal(cf[:, 4:5], cf[:, 3:4])
                     A = meta.tile([P, 1], F32, name=f"A{l}")
                     B = meta.tile([P, 1], F32, name=f"B{l}")
                     nc.vector.tensor_tensor(out=A[:], in0=g_sb[l][:], in1=cf[:, 4:5], op=Alu.mult)
                     nc.vector.tensor_tensor(out=cf[:, 5:6], in0=cf[:, 0:1], in1=A[:], op=Alu.mult)
                     nc.vector.tensor_tensor(out=B[:], in0=bt_sb[l][:], in1=cf[:, 5:6], op=Alu.subtract)
                     if STAGE == 2:
                         nc.vector.tensor_copy(out_store[:1, 0:1], B[:1, :])
                         break

                     # ---- epilogue: act, transpose to node-major, store table slice ----
                     for t in range(NT):
                         d_hi = LAST_D if t == NT - 1 else P
                         act = actp.tile([P, P], F32, tag="act")
                         nc.scalar.activation(out=act[:, :d_hi], in_=ystore[:, t * P:t * P + d_hi],
                                              func=Act.Relu, bias=B[:], scale=A[:])
                         tr = trps_p.tile([P, P], F32, tag="tr")
                         nc.tensor.transpose(tr[:d_hi, :], act[:, :d_hi], ident[:])
                         h = hp.tile([P, HID], DT_TAB, tag="h")
                         nc.vector.tensor_scalar_mul(h[:d_hi, :], tr[:d_hi, :], dinv_sl[:d_hi, t:t + 1])
                         nc.sync.dma_start(tab_in[l][t * P:t * P + d_hi, :], h[:d_hi, :])
                     nc.gpsimd.collective_compute(
                         "AllGather", Alu.bypass, replica_groups=[list(range(NCORES))],
                         ins=[tab_in[l][:]], outs=[tab_out[l][:]],
                     )
                     if STAGE == 3 and l == 0:
                         hh = hp.tile([P, HID], DT_TAB, tag="h")
                         nc.sync.dma_start(hh[:], tab_out[0][:P, :])
                         nc.vector.tensor_copy(out_store[:1, :P], hh[:1, :])
                         break
                     if STAGE == 4 and l == 1:
                         nc.vector.tensor_copy(out_store[:1, :P], ystore[:1, :P])
                         break

            nc.sync.dma_start(outv[:], out_store[:])

    nc.compile()
    return nc


def _prep(inputs):
    x = np.asarray(inputs["x"], np.float32)
    ei = np.asarray(inputs["edge_index"], np.int64)
    loops = np.arange(N, dtype=np.int64)
    src = np.concatenate([ei[0], loops])
    dst = np.concatenate([ei[1], loops])
    deg = np.bincount(dst, minlength=N).astype(np.float32)
    dinv = (1.0 / np.sqrt(deg)).astype(np.float32)
    xs = (x * dinv[:, None]).astype(NP_TAB)

    core = dst // NPC
    rem = dst - core * NPC
    tidx = rem >> 7
    order = np.lexsort((tidx, core))
    src_s = src[order].astype(np.int32)
    core_s = core[order]
    tidx_s = tidx[order]
    loc_s = (rem[order] & 127).astype(np.float32)

    gk = core_s * NT + tidx_s
    cnt = np.bincount(gk, minlength=NCORES * NT).reshape(NCORES, NT)
    S = np.maximum(np.ceil(cnt.max(axis=0) / P).astype(np.int64), 1)
    T = int(S.sum())
    off = np.zeros(NT, np.int64)
    off[1:] = np.cumsum(S)[:-1]

    starts = np.zeros(NCORES * NT, np.int64)
    starts[1:] = np.cumsum(cnt.reshape(-1))[:-1]
    pos = np.arange(len(src_s)) - starts[gk]
    sub = pos >> 7
    lane = pos & 127
    col = off[tidx_s] + sub

    gidx = np.zeros((NCORES, P, T), np.int32)
    dloc = np.full((NCORES, P, T), 1000.0, np.float32)
    gidx[core_s, lane, col] = src_s
    dloc[core_s, lane, col] = loc_s

    dinv_pad = np.zeros(NCORES * NT * P, np.float32)
    dv = dinv.reshape(NCORES, NPC)
    dinv_pad = np.zeros((NCORES, NT * P), np.float32)
    dinv_pad[:, :NPC] = dv
    dinv_sl = dinv_pad.reshape(NCORES, NT, P).transpose(0, 2, 1).copy()  # [c, P, NT]

    com = {
        "xs": np.ascontiguousarray(xs),
        "W1": np.asarray(inputs["W1"], np.float32),
        "W2": np.asarray(inputs["W2"], np.float32),
        "W3": np.asarray(inputs["W3"], np.float32),
        "fcW": np.asarray(inputs["fcW"], np.float32).reshape(HID, 1),
        "g1": np.asarray(inputs["g1"], np.float32).reshape(HID, 1),
        "g2": np.asarray(inputs["g2"], np.float32).reshape(HID, 1),
        "bt1": np.asarray(inputs["bt1"], np.float32).reshape(HID, 1),
        "bt2": np.asarray(inputs["bt2"], np.float32).reshape(HID, 1),
        "b3": np.asarray(inputs["b3"], np.float32).reshape(HID, 1),
    }
    in_maps = []
    for c in range(NCORES):
        m = dict(com)
        m["gidx"] = np.ascontiguousarray(gidx[c])
        m["dloc"] = np.ascontiguousarray(dloc[c])
        m["dinv_sl"] = np.ascontiguousarray(dinv_sl[c])
        in_maps.append(m)
    return in_maps, T, tuple(int(s) for s in S)


def _get_nc(T, S):
    key = (T, S, BF16_TABLES, STAGE, REPS, NOCOLL, AGLOCAL, SFRAC, SKIPW)
    if key not in _NC_CACHE:
        _NC_CACHE[key] = _build(T, list(S))
    return _NC_CACHE[key]


class _Exec:
    """jit-once / device_put-once executor mirroring bass2jax.run_bass_via_pjrt."""

    def __init__(self, nc, in_maps):
        import jax
        from jax.sharding import Mesh, PartitionSpec
        from jax.experimental.shard_map import shard_map
        from concourse import bass2jax
        bass2jax.install_neuronx_cc_hook()
        n_cores = NCORES
        part_name = nc.partition_id_tensor.name if nc.partition_id_tensor else None
        in_names, out_names, out_avals, zero_outs = [], [], [], []
        for alloc in nc.m.functions[0].allocations:
            if not isinstance(alloc, mybir.MemoryLocationSet):
                continue
            name = alloc.memorylocations[0].name
            if alloc.kind == "ExternalInput":
                if name != part_name:
                    in_names.append(name)
            elif alloc.kind == "ExternalOutput":
                out_names.append(name)
                shape = tuple(alloc.tensor_shape)
                dtype = mybir.dt.np(alloc.dtype)
                out_avals.append(jax.core.ShapedArray(shape, dtype))
                zero_outs.append(np.zeros(shape, dtype))
        n_params = len(in_names)
        all_names = in_names + out_names
        if part_name is not None:
            all_names = all_names + [part_name]
        self.out_names, self.out_avals, self.n_cores = out_names, out_avals, n_cores

        def _body(*args):
            operands = list(args)
            if part_name is not None:
                operands.append(bass2jax.partition_id_tensor())
            outs = bass2jax._bass_exec_p.bind(
                *operands,
                out_avals=tuple(out_avals),
                in_names=tuple(all_names),
                out_names=tuple(out_names),
                lowering_input_output_aliases=(),
                sim_require_finite=True,
                sim_require_nnan=True,
                nc=nc,
            )
            return tuple(outs)

        devices = jax.devices()[:n_cores]
        mesh = Mesh(np.asarray(devices), ("core",))
        in_specs = (PartitionSpec("core"),) * (n_params + len(out_names))
        out_specs = (PartitionSpec("core"),) * len(out_names)
        self.fn = jax.jit(
            shard_map(_body, mesh=mesh, in_specs=in_specs, out_specs=out_specs,
                      check_rep=False),
            keep_unused=True,
        )
        concat_in = [
            np.concatenate([np.asarray(in_maps[c][k]) for c in range(n_cores)], axis=0)
            for k in in_names
        ]
        concat_zeros = [
            np.zeros((n_cores * z.shape[0], *z.shape[1:]), z.dtype) for z in zero_outs
        ]
        sh = jax.sharding.NamedSharding(mesh, PartitionSpec("core"))
        self.dev_in = [jax.device_put(a, sh) for a in concat_in] + \
                      [jax.device_put(a, sh) for a in concat_zeros]
        for a in self.dev_in:
            a.block_until_ready()

    def run(self):
        outs = self.fn(*self.dev_in)
        for o in outs:
            o.block_until_ready()
        return outs

    def results(self):
        outs = self.run()
        res = [dict() for _ in range(self.n_cores)]
        for i, name in enumerate(self.out_names):
            arr = np.asarray(outs[i]).reshape(self.n_cores, *self.out_avals[i].shape)
            for c in range(self.n_cores):
                res[c][name] = arr[c]
        return res


_EXEC_CACHE = {}


def _get_exec(in_maps, T, S):
    key = (T, S, BF16_TABLES, STAGE, REPS, NOCOLL, AGLOCAL, SFRAC, SKIPW)
    if key not in _EXEC_CACHE:
        _EXEC_CACHE[key] = _Exec(_get_nc(T, S), in_maps)
    return _EXEC_CACHE[key]


def _run(in_maps, T, S):
    nc = _get_nc(T, S)
    r = bass_utils.run_bass_kernel_spmd(nc, in_maps, core_ids=list(range(NCORES)), trace=False)
    return r


def kernel(**inputs):
    in_maps, T, S = _prep(inputs)
    r = _run(in_maps, T, S)
    out = np.concatenate([r.results[c]["outv"].reshape(-1) for c in range(NCORES)])
    fcb = np.asarray(inputs["fcb"], np.float32).reshape(-1)
    out = (out + fcb[0]).astype(np.float32)[:, None]
    # numerically stable sigmoid in fp32
    sig = np.empty_like(out)
    pos = out >= 0
    sig[pos] = 1.0 / (1.0 + np.exp(-out[pos], dtype=np.float32))
    ex = np.exp(out[~pos], dtype=np.float32)
    sig[~pos] = ex / (1.0 + ex)
    return out, sig



# revision 22
# speedup vs baseline: 4.3722x; 4.3722x over previous
"""BindingPocketGNN (3-layer GCN, N=50000, E=800000) on 8 Trainium2 NeuronCores.

Distribution: nodes sharded into 8 contiguous ranges (6250/core). Each core owns the
scatter/aggregation for its destination-node range; edges are routed (host-side) to the
core owning their destination. Source features are gathered from a replicated
node-major table (input x for layer 1; AllGather-replicated activations for layers 2/3)
with the ant-custom SWDGE dma_gather (int16 indices; table split in two halves at row
32768 so indices fit; per-group one gather per half).

Per layer, on each core (feat-major formulation so BN/bias are per-partition):
    z^T[f_in, d]  = sum_e  msg_e[f_in] * mask[e, d]      (TensorE: msg.T @ (iota==dst) mask)
    zs            = z^T * dinv[dst]                      (fused in PSUM->SBUF copy)
    y^T[f_out, d] = W.T @ zs                             (TensorE)
    stats         = AllReduce(sum/sumsq of y)            (1KB collective; BN layers)
    act^T         = Relu(A*y^T + B)                      (ScalarE, per-partition A/B)
    h             = act^T transposed to node-major       (TensorE transpose)
    table_{l+1}   = AllGather(h * dinv)                  (collective; layers 1,2)
Layer 3 feeds a [128->1] FC matmul; +fcb and sigmoid applied on host.

Edges are packed host-side into 128-lane subtiles per (dst tile, table half); GRP
consecutive dst tiles form a gather group: [A-block subtiles][B-block subtiles], one
dma_gather per (group, half), one batched one-hot mask build per group.
"""
import sys
if "/opt/trn_rl_repo" not in sys.path:
    sys.path.insert(0, "/opt/trn_rl_repo")

import os
import numpy as np
import ml_dtypes

import concourse.bass as bass
import concourse.bacc as bacc
import concourse.mybir as mybir
import concourse.tile as tile
from concourse import bass_utils, library_config
from concourse.masks import make_identity

N = 50000
E = 800000
IN, HID = 64, 128
BN_EPS = 1e-5
NCORES = 8
NPC = N // NCORES          # 6250 nodes per core
P = 128
NT = (NPC + P - 1) // P    # 49 dst tiles per core
LAST_D = NPC - (NT - 1) * P  # 106
NPCP = NT * P              # per-core table rows padded to tile multiple (6272)
NROW = NPCP * NCORES       # padded global table rows (50176)
HALF = 32768               # int16 index limit: table split at this (padded) row

F32 = mybir.dt.float32
I16 = mybir.dt.int16
BF16 = mybir.dt.bfloat16
FP8E4 = mybir.dt.float8e4
DT_TAB = BF16
NP_TAB = np.dtype(ml_dtypes.bfloat16)

REPS = int(os.environ.get("GCN_REPS", "1"))
SIMNC = os.environ.get("GCN_SIMNC", "0") == "1"  # single-core: stub collectives with local DMA
GRP = int(os.environ.get("GCN_GRP", "7"))        # dst tiles per gather group
NQ = int(os.environ.get("GCN_NQ", "4"))         # SWDGE queues (1-4)
CHUNK = int(os.environ.get("GCN_CHUNK", "8"))   # subtiles per dma_gather (<=8: 1024-idx ring limit)
FP8 = os.environ.get("GCN_FP8", "0") == "1"      # fp8e4 tables for layers 2,3 (too lossy: 5e-2)

Alu = mybir.AluOpType
Act = mybir.ActivationFunctionType

_NC_CACHE = {}


def _dma_gather_raw(gp, out_ap, in_ap, idxs_ap, num_idxs, elem_size,
                    elem_step=None, queue_num=0, single_packet=True):
    """bass.dma_gather minus the transpose-only %256 elem_size assert.
    Non-transpose DRAM-source gather; row stride must still be 256B-aligned."""
    from concourse import ap_utils
    from concourse.bass import exact_div, round_up_to_multiple
    assert idxs_ap.dtype == mybir.dt.int16
    assert in_ap.space == bass.MemorySpace.DRAM
    assert idxs_ap.space == bass.MemorySpace.SBUF
    assert out_ap.space == bass.MemorySpace.SBUF
    assert in_ap.dtype == out_ap.dtype
    if elem_step is None:
        assert ap_utils.ap_is_contiguous(in_ap.ap[1:])
        elem_step = elem_size
    assert ap_utils.ap_is_contiguous(out_ap.ap[1:])
    assert ap_utils.ap_is_contiguous(idxs_ap.ap[1:])
    assert in_ap.ap[-1][1] == elem_size
    assert out_ap.ap[-1][1] == elem_size
    assert out_ap.ap[0][1] * out_ap.ap[1][1] == round_up_to_multiple(num_idxs, 128)
    assert in_ap.ap[0][0] == elem_step
    stride_bytes = elem_step * mybir.dt.size(in_ap.dtype)
    stride_bytes_256 = exact_div(stride_bytes, 256)
    assert stride_bytes_256 < 256
    _in_ap = gp.lower_ap_dma(in_ap, for_custom_bir_dma=True)
    _idxs_ap = gp.lower_ap(idxs_ap)
    _out_ap = gp.lower_ap(out_ap)
    return gp.add_instruction(
        mybir.InstDMAGatherAnt(
            name=gp.bass.get_next_instruction_name(),
            ins=[*_in_ap, _idxs_ap, gp.lower_val_access(gp.to_reg(num_idxs))],
            outs=[_out_ap],
            transpose=False,
            num_idxs=num_idxs,
            elem_size=elem_size,
            stride_bytes_256=stride_bytes_256,
            gen_mode=0,
            single_packet=single_packet,
            queue_num=queue_num,
            sbuf_tokens_per_rank=0,
            sbuf_free_dim_per_rank=0,
            sbuf_free_dim_pad_per_rank=0,
            sbuf_byte_offset=0,
        )
    )


def _mk_groups(S0, S1):
    """Column layout: per group of GRP tiles, [A-block per tile][B-block per tile].
    Returns (T, groups) with groups = list of dicts."""
    groups = []
    colptr = 0
    for g0 in range(0, NT, GRP):
        tiles = list(range(g0, min(g0 + GRP, NT)))
        colA = colptr
        astart = {}
        for t in tiles:
            astart[t] = colptr
            colptr += S0[t]
        colB = colptr
        bstart = {}
        for t in tiles:
            bstart[t] = colptr
            colptr += S1[t]
        groups.append(dict(tiles=tiles, colA=colA, colB=colB, colEnd=colptr,
                           astart=astart, bstart=bstart))
    return colptr, groups


def _build(S0, S1):
    """Build+schedule the SPMD program. S0/S1: per-dst-tile subtile counts for
    table halves A/B (len NT each, identical across cores)."""
    T, groups = _mk_groups(S0, S1)
    SgMax = max(g["colEnd"] - g["colA"] for g in groups)

    nc = bacc.Bacc("TRN2", target_bir_lowering=False, debug=False, num_devices=NCORES,
                   num_swdge_queues=NQ)

    # ---- I/O ----
    xs = nc.dram_tensor("xs", [NROW, P], DT_TAB, kind="ExternalInput")  # padded rows+cols
    gidx_d = nc.dram_tensor("gidx16", [P, 8 * T], I16, kind="ExternalInput")
    dloc_d = nc.dram_tensor("dloc", [P, T], F32, kind="ExternalInput")
    dinv_d = nc.dram_tensor("dinv_sl", [P, NT], F32, kind="ExternalInput")
    W_d = [
        nc.dram_tensor("W1", [IN, HID], F32, kind="ExternalInput"),
        nc.dram_tensor("W2", [HID, HID], F32, kind="ExternalInput"),
        nc.dram_tensor("W3", [HID, HID], F32, kind="ExternalInput"),
    ]
    fcW_d = nc.dram_tensor("fcW", [HID, 1], F32, kind="ExternalInput")
    g_d = [nc.dram_tensor("g1", [HID, 1], F32, kind="ExternalInput"),
           nc.dram_tensor("g2", [HID, 1], F32, kind="ExternalInput")]
    bt_d = [nc.dram_tensor("bt1", [HID, 1], F32, kind="ExternalInput"),
            nc.dram_tensor("bt2", [HID, 1], F32, kind="ExternalInput")]
    b3_d = nc.dram_tensor("b3", [HID, 1], F32, kind="ExternalInput")
    outv = nc.dram_tensor("outv", [1, NPC], F32, kind="ExternalOutput")

    with tile.TileContext(nc) as tc:
        with (
            tc.tile_pool(name="meta", bufs=1) as meta,
            tc.tile_pool(name="msgp16", bufs=2) as msgp16,
            tc.tile_pool(name="msgp8", bufs=2) as msgp8,
            tc.tile_pool(name="maskp", bufs=2) as maskp,
            tc.tile_pool(name="zsp", bufs=3) as zsp,
            tc.tile_pool(name="actp", bufs=3) as actp,
            tc.tile_pool(name="hp", bufs=3) as hp,
            tc.tile_pool(name="sqp", bufs=2) as sqp,
            tc.tile_pool(name="zps_p", bufs=2, space="PSUM") as zps_p,
            tc.tile_pool(name="yps_p", bufs=2, space="PSUM") as yps_p,
            tc.tile_pool(name="trps_p", bufs=2, space="PSUM") as trps_p,
            tc.tile_pool(name="fcps_p", bufs=1, space="PSUM") as fcps_p,
            tc.tile_pool(name="dram", bufs=1, space="DRAM") as dram,
        ):
            # ---- resident metadata ----
            gidx_sb = meta.tile([P, 8 * T], I16)
            nc.sync.dma_start(gidx_sb[:], gidx_d[:])
            dloc_sb = meta.tile([P, T], F32)
            nc.sync.dma_start(dloc_sb[:], dloc_d[:])
            dinv_sl = meta.tile([P, NT], F32)
            nc.sync.dma_start(dinv_sl[:], dinv_d[:])
            W_sb = []
            for l in range(3):
                fi = IN if l == 0 else HID
                wf = meta.tile([fi, HID], F32, name=f"W{l}_f32")
                nc.sync.dma_start(wf[:], W_d[l][:])
                w = meta.tile([fi, HID], BF16, name=f"W{l}_sb")
                nc.vector.tensor_copy(w[:], wf[:])
                W_sb.append(w)
            fcW_sb = meta.tile([HID, 1], F32)
            nc.sync.dma_start(fcW_sb[:], fcW_d[:])
            g_sb, bt_sb = [], []
            for l in range(2):
                gg = meta.tile([HID, 1], F32, name=f"g{l}_sb")
                nc.sync.dma_start(gg[:], g_d[l][:])
                g_sb.append(gg)
                bb = meta.tile([HID, 1], F32, name=f"bt{l}_sb")
                nc.sync.dma_start(bb[:], bt_d[l][:])
                bt_sb.append(bb)
            b3_sb = meta.tile([HID, 1], F32)
            nc.sync.dma_start(b3_sb[:], b3_d[:])
            eps_sb = meta.tile([P, 1], F32)
            nc.vector.memset(eps_sb[:], BN_EPS)

            ident = meta.tile([P, P], F32)
            make_identity(nc, ident[:])
            iota_i = meta.tile([P, P], mybir.dt.int32)
            nc.gpsimd.iota(iota_i[:], pattern=[[1, P]], base=0, channel_multiplier=0)
            iota_t = meta.tile([P, P], DT_TAB)
            nc.vector.tensor_copy(iota_t[:], iota_i[:])

            # dinv broadcast rows: dinv_bc[:, t*128+j] = dinv of node t*128+j
            dinv_bc = meta.tile([P, NT * P], BF16)
            for t in range(NT):
                tr = trps_p.tile([P, P], F32, tag="tr")
                nc.tensor.transpose(tr[:], dinv_sl[:, t:t + 1].to_broadcast([P, P]), ident[:])
                nc.vector.tensor_copy(dinv_bc[:, t * P:(t + 1) * P], tr[:])

            mask_dram = dram.tile([P, T * P], FP8E4, name="mask_cache")
            ystore = meta.tile([P, NT * P], BF16)
            sums = meta.tile([P, NT], F32)
            sumsq = meta.tile([P, NT], F32)
            out_store = meta.tile([1, NPC], F32)

            # internal DRAM for collectives (fresh per rep: Shared allows one writer)
            TW = 2 * HID if FP8 else HID       # fp8 rows padded to 256B stride
            DT_T = FP8E4 if FP8 else DT_TAB
            def mk_coll(rep):
                tab_in = [dram.tile([NPCP, TW], DT_T, name=f"tab{l}_in_r{rep}") for l in (1, 2)]
                tab_out = [dram.tile([NROW, TW], DT_T, name=f"tab{l}_out_r{rep}",
                                     addr_space="Local" if SIMNC else "Shared")
                           for l in (1, 2)]
                st_in = [dram.tile([P, 2], F32, name=f"st{l}_in_r{rep}") for l in (0, 1)]
                st_out = [dram.tile([P, 2], F32, name=f"st{l}_out_r{rep}",
                                    addr_space="Local" if SIMNC else "Shared")
                          for l in (0, 1)]
                return tab_in, tab_out, st_in, st_out

            def allreduce_stats(st_in_t, st_out_t):
                if SIMNC:
                    nc.sync.dma_start(st_out_t[:], st_in_t[:])
                else:
                    nc.gpsimd.collective_compute(
                        "AllReduce", Alu.add, replica_groups=[list(range(NCORES))],
                        ins=[st_in_t[:]], outs=[st_out_t[:]],
                    )

            def allgather_tab(tab_in_t, tab_out_t):
                if SIMNC:
                    for c in range(NCORES):
                        nc.sync.dma_start(tab_out_t[c * NPCP:(c + 1) * NPCP, :], tab_in_t[:])
                else:
                    nc.gpsimd.collective_compute(
                        "AllGather", Alu.bypass, replica_groups=[list(range(NCORES))],
                        ins=[tab_in_t[:]], outs=[tab_out_t[:]],
                    )

            qrr = [0]
            for _rep in range(REPS):
              tab_in, tab_out, st_in, st_out = mk_coll(_rep)
              for l in range(3):
                 f_in = IN if l == 0 else HID
                 dt_l = FP8E4 if (FP8 and l > 0) else DT_TAB
                 if l == 0:
                     tabA, tabB = xs[0:HALF, 0:IN], xs[HALF:NROW, 0:IN]
                 else:
                     # fp8 tables are [NROW, 256] with payload in cols 0:128 (256B stride)
                     tabA = tab_out[l - 1][0:HALF, 0:HID]
                     tabB = tab_out[l - 1][HALF:NROW, 0:HID]
                 # ---- aggregation + weight matmul ----
                 for g in groups:
                     colA, colB, colEnd = g["colA"], g["colB"], g["colEnd"]
                     SgA, SgB = colB - colA, colEnd - colB
                     Sg = SgA + SgB
                     if l == 0:
                         msg = msgp16.tile([P, SgMax * f_in], dt_l, tag="msg16")
                     else:
                         msg = (msgp8 if FP8 else msgp16).tile(
                             [P, SgMax * f_in], dt_l, tag="msg8" if FP8 else "msg16")
                     step_l = P if l == 0 else (2 * HID if FP8 else HID)

                     def chunked_gather(col0, col1, tab):
                         for c0 in range(col0, col1, CHUNK):
                             c1 = min(c0 + CHUNK, col1)
                             k = c1 - c0
                             _dma_gather_raw(
                                 nc.gpsimd,
                                 out_ap=msg[:, (c0 - colA) * f_in:(c1 - colA) * f_in]
                                     .rearrange("p (s j) -> p s j", s=k),
                                 in_ap=tab, idxs_ap=gidx_sb[:, 8 * c0:8 * c1],
                                 num_idxs=k * P, elem_size=f_in, elem_step=step_l,
                                 queue_num=qrr[0] % NQ,
                             )
                             qrr[0] += 1

                     if SgA:
                         chunked_gather(colA, colB, tabA)
                     if SgB:
                         chunked_gather(colB, colEnd, tabB)
                     mask = maskp.tile([P, SgMax * P], FP8E4, tag="mask")
                     if _rep == 0 and l == 0:
                         nc.vector.tensor_tensor(
                             out=mask[:, :Sg * P].rearrange("p (s j) -> p s j", s=Sg),
                             in0=iota_t[:].unsqueeze(1).broadcast_to([P, Sg, P]),
                             in1=dloc_sb[:, colA:colEnd].to_broadcast([P, Sg, P]),
                             op=Alu.is_equal,
                         )
                         nc.sync.dma_start(mask_dram[:, colA * P:colEnd * P], mask[:, :Sg * P])
                     else:
                         nc.sync.dma_start(mask[:, :Sg * P], mask_dram[:, colA * P:colEnd * P])
                     for t in g["tiles"]:
                         d_hi = LAST_D if t == NT - 1 else P
                         subcols = (list(range(g["astart"][t], g["astart"][t] + S0[t])) +
                                    list(range(g["bstart"][t], g["bstart"][t] + S1[t])))
                         zps = zps_p.tile([P, P], F32, tag="zps")
                         for i, c in enumerate(subcols):
                             cl = c - colA
                             nc.tensor.matmul(zps[:f_in, :],
                                              lhsT=msg[:, cl * f_in:(cl + 1) * f_in],
                                              rhs=mask[:, cl * P:(cl + 1) * P],
                                              start=(i == 0), stop=(i == len(subcols) - 1))
                         zs = zsp.tile([P, P], BF16, tag="zs")
                         nc.vector.tensor_tensor(
                             out=zs[:f_in, :], in0=zps[:f_in, :],
                             in1=dinv_bc[:f_in, t * P:(t + 1) * P], op=Alu.mult,
                         )
                         yps = yps_p.tile([P, P], F32, tag="yps")
                         nc.tensor.matmul(yps[:], lhsT=W_sb[l][:], rhs=zs[:f_in, :],
                                          start=True, stop=True)
                         if l < 2:
                             nc.scalar.activation(
                                 out=ystore[:, t * P:t * P + d_hi], in_=yps[:, :d_hi],
                                 func=Act.Copy, accum_out=sums[:, t:t + 1],
                             )
                             sq = sqp.tile([P, P], F32, tag="sq")
                             nc.scalar.activation(
                                 out=sq[:, :d_hi], in_=yps[:, :d_hi],
                                 func=Act.Square, accum_out=sumsq[:, t:t + 1],
                             )
                         else:
                             act3 = actp.tile([P, P], F32, tag="act")
                             nc.scalar.activation(out=act3[:, :d_hi], in_=yps[:, :d_hi],
                                                  func=Act.Relu, bias=b3_sb[:], scale=1.0)
                             fcp = fcps_p.tile([1, P], F32, tag="fcp")
                             nc.tensor.matmul(fcp[:1, :d_hi], lhsT=fcW_sb[:], rhs=act3[:, :d_hi],
                                              start=True, stop=True)
                             nc.vector.tensor_copy(out_store[:1, t * P:t * P + d_hi], fcp[:1, :d_hi])

                 if l < 2:
                     # ---- BN stats allreduce + coefficients ----
                     stats = meta.tile([P, 2], F32, name=f"stats{l}_r{_rep}")
                     nc.vector.tensor_reduce(stats[:, 0:1], sums[:], axis=mybir.AxisListType.X, op=Alu.add)
                     nc.vector.tensor_reduce(stats[:, 1:2], sumsq[:], axis=mybir.AxisListType.X, op=Alu.add)
                     nc.sync.dma_start(st_in[l][:], stats[:])
                     allreduce_stats(st_in[l], st_out[l])
                     tot = meta.tile([P, 2], F32, name=f"tot{l}_r{_rep}")
                     nc.sync.dma_start(tot[:], st_out[l][:])
                     cf = meta.tile([P, 6], F32, name=f"cf{l}_r{_rep}")  # mean ex2 var std A B
                     nc.vector.tensor_scalar_mul(cf[:, 0:1], tot[:, 0:1], 1.0 / N)
                     nc.vector.tensor_scalar_mul(cf[:, 1:2], tot[:, 1:2], 1.0 / N)
                     nc.vector.tensor_tensor(out=cf[:, 2:3], in0=cf[:, 0:1], in1=cf[:, 0:1], op=Alu.mult)
                     nc.vector.tensor_tensor(out=cf[:, 2:3], in0=cf[:, 1:2], in1=cf[:, 2:3], op=Alu.subtract)
                     nc.scalar.activation(out=cf[:, 3:4], in_=cf[:, 2:3], func=Act.Sqrt, bias=eps_sb[:], scale=1.0)
                     nc.vector.reciprocal(cf[:, 4:5], cf[:, 3:4])
                     A = meta.tile([P, 1], F32, name=f"A{l}_r{_rep}")
                     B = meta.tile([P, 1], F32, name=f"B{l}_r{_rep}")
                     nc.vector.tensor_tensor(out=A[:], in0=g_sb[l][:], in1=cf[:, 4:5], op=Alu.mult)
                     nc.vector.tensor_tensor(out=cf[:, 5:6], in0=cf[:, 0:1], in1=A[:], op=Alu.mult)
                     nc.vector.tensor_tensor(out=B[:], in0=bt_sb[l][:], in1=cf[:, 5:6], op=Alu.subtract)

                     # ---- epilogue: act, transpose to node-major, store table slice ----
                     hstore = meta.tile([P, NT * HID], DT_T, name=f"hstore{l}")
                     for t in range(NT):
                         d_hi = LAST_D if t == NT - 1 else P
                         act = actp.tile([P, P], F32, tag="act")
                         nc.scalar.activation(out=act[:, :d_hi], in_=ystore[:, t * P:t * P + d_hi],
                                              func=Act.Relu, bias=B[:], scale=A[:])
                         tr = trps_p.tile([P, P], F32, tag="tr")
                         nc.tensor.transpose(tr[:d_hi, :], act[:, :d_hi], ident[:])
                         nc.vector.tensor_scalar_mul(hstore[:d_hi, t * HID:(t + 1) * HID],
                                                     tr[:d_hi, :], dinv_sl[:d_hi, t:t + 1])
                     # one DMA: hstore [128, NT, HID] -> tab_in rows (t*128+r)
                     nc.sync.dma_start(
                         tab_in[l][:].rearrange("(t r) f -> r t f", r=P)[:, :, 0:HID],
                         hstore[:].rearrange("p (t f) -> p t f", f=HID),
                     )
                     allgather_tab(tab_in[l], tab_out[l])

            nc.sync.dma_start(outv[:], out_store[:])

    nc.compile()
    return nc


def _prep(inputs):
    x = np.asarray(inputs["x"], np.float32)
    ei = np.asarray(inputs["edge_index"], np.int64)
    loops = np.arange(N, dtype=np.int64)
    src = np.concatenate([ei[0], loops])
    dst = np.concatenate([ei[1], loops])
    deg = np.bincount(dst, minlength=N).astype(np.float32)
    dinv = (1.0 / np.sqrt(deg)).astype(np.float32)
    all_pr = (np.arange(N) // NPC) * NPCP + (np.arange(N) % NPC)  # node -> padded row
    xs_pad = np.zeros((NROW, P), NP_TAB)
    xs_pad[all_pr, :IN] = (x * dinv[:, None]).astype(NP_TAB)

    core = dst // NPC
    rem = dst - core * NPC
    tidx = rem >> 7
    srcp = all_pr[src]                     # padded-space source rows
    half = (srcp >= HALF).astype(np.int64)

    # per-(core, tile, half) counts -> shared subtile shape S0/S1 (max over cores)
    key = (core * NT + tidx) * 2 + half
    cnt = np.bincount(key, minlength=NCORES * NT * 2).reshape(NCORES, NT, 2)
    Sh = np.ceil(cnt.max(axis=0) / P).astype(np.int64)  # [NT, 2]
    S0, S1 = tuple(int(v) for v in Sh[:, 0]), tuple(int(v) for v in Sh[:, 1])

    # column layout identical to _mk_groups
    T, groups = _mk_groups(S0, S1)
    colstart = np.zeros((NT, 2), np.int64)
    for g in groups:
        for t in g["tiles"]:
            colstart[t, 0] = g["astart"][t]
            colstart[t, 1] = g["bstart"][t]

    # position of each edge within its (core, tile, half) block
    order = np.lexsort((half, tidx, core))
    src_s = srcp[order]
    core_s = core[order]
    key_s = key[order]
    loc_s = (rem[order] & 127).astype(np.float32)
    half_s = half[order]
    tidx_s = tidx[order]

    starts = np.zeros(NCORES * NT * 2, np.int64)
    starts[1:] = np.cumsum(cnt.reshape(-1))[:-1]
    pos = np.arange(len(src_s)) - starts[key_s]
    sub = pos >> 7
    lane = (pos & 127).astype(np.int64)
    col = colstart[tidx_s, half_s] + sub          # global subtile column

    # dloc [core, 128, T]
    dloc = np.full((NCORES, P, T), 1000.0, np.float32)
    dloc[core_s, lane, col] = loc_s

    # gidx16 [core, 16, 8T] -> tiled to 128 partitions
    # flat gather position within (group, half) block: k = (col-colH)*128+lane;
    # wrap target: partition = lane & 15, column16 = col*8 + (lane >> 4)
    idx16 = np.zeros((NCORES, 16, 8 * T), np.int16)
    rebased = (src_s - half_s * HALF).astype(np.int16)
    idx16[core_s, lane & 15, col * 8 + (lane >> 4)] = rebased
    gidx16 = np.tile(idx16, (1, 8, 1))

    dinv_pad = np.zeros((NCORES, NT * P), np.float32)
    dinv_pad[:, :NPC] = dinv.reshape(NCORES, NPC)
    dinv_sl = dinv_pad.reshape(NCORES, NT, P).transpose(0, 2, 1).copy()  # [c, P, NT]

    com = {
        "xs": np.ascontiguousarray(xs_pad),
        "W1": np.asarray(inputs["W1"], np.float32),
        "W2": np.asarray(inputs["W2"], np.float32),
        "W3": np.asarray(inputs["W3"], np.float32),
        "fcW": np.asarray(inputs["fcW"], np.float32).reshape(HID, 1),
        "g1": np.asarray(inputs["g1"], np.float32).reshape(HID, 1),
        "g2": np.asarray(inputs["g2"], np.float32).reshape(HID, 1),
        "bt1": np.asarray(inputs["bt1"], np.float32).reshape(HID, 1),
        "bt2": np.asarray(inputs["bt2"], np.float32).reshape(HID, 1),
        "b3": np.asarray(inputs["b3"], np.float32).reshape(HID, 1),
    }
    in_maps = []
    for c in range(NCORES):
        m = dict(com)
        m["gidx16"] = np.ascontiguousarray(gidx16[c])
        m["dloc"] = np.ascontiguousarray(dloc[c])
        m["dinv_sl"] = np.ascontiguousarray(dinv_sl[c])
        in_maps.append(m)
    return in_maps, S0, S1


def _get_nc(S0, S1):
    key = (S0, S1, REPS, SIMNC, GRP, FP8, NQ, CHUNK)
    if key not in _NC_CACHE:
        _NC_CACHE[key] = _build(list(S0), list(S1))
    return _NC_CACHE[key]


class _Exec:
    """jit-once / device_put-once executor mirroring bass2jax.run_bass_via_pjrt."""

    def __init__(self, nc, in_maps):
        import jax
        from jax.sharding import Mesh, PartitionSpec
        from jax.experimental.shard_map import shard_map
        from concourse import bass2jax
        bass2jax.install_neuronx_cc_hook()
        n_cores = NCORES
        part_name = nc.partition_id_tensor.name if nc.partition_id_tensor else None
        in_names, out_names, out_avals, zero_outs = [], [], [], []
        for alloc in nc.m.functions[0].allocations:
            if not isinstance(alloc, mybir.MemoryLocationSet):
                continue
            name = alloc.memorylocations[0].name
            if alloc.kind == "ExternalInput":
                if name != part_name:
                    in_names.append(name)
            elif alloc.kind == "ExternalOutput":
                out_names.append(name)
                shape = tuple(alloc.tensor_shape)
                dtype = mybir.dt.np(alloc.dtype)
                out_avals.append(jax.core.ShapedArray(shape, dtype))
                zero_outs.append(np.zeros(shape, dtype))
        n_params = len(in_names)
        all_names = in_names + out_names
        if part_name is not None:
            all_names = all_names + [part_name]
        self.out_names, self.out_avals, self.n_cores = out_names, out_avals, n_cores

        def _body(*args):
            operands = list(args)
            if part_name is not None:
                operands.append(bass2jax.partition_id_tensor())
            outs = bass2jax._bass_exec_p.bind(
                *operands,
                out_avals=tuple(out_avals),
                in_names=tuple(all_names),
                out_names=tuple(out_names),
                lowering_input_output_aliases=(),
                sim_require_finite=True,
                sim_require_nnan=True,
                nc=nc,
            )
            return tuple(outs)

        devices = jax.devices()[:n_cores]
        mesh = Mesh(np.asarray(devices), ("core",))
        in_specs = (PartitionSpec("core"),) * (n_params + len(out_names))
        out_specs = (PartitionSpec("core"),) * len(out_names)
        self.fn = jax.jit(
            shard_map(_body, mesh=mesh, in_specs=in_specs, out_specs=out_specs,
                      check_rep=False),
            keep_unused=True,
        )
        concat_in = [
            np.concatenate([np.asarray(in_maps[c][k]) for c in range(n_cores)], axis=0)
            for k in in_names
        ]
        concat_zeros = [
            np.zeros((n_cores * z.shape[0], *z.shape[1:]), z.dtype) for z in zero_outs
        ]
        sh = jax.sharding.NamedSharding(mesh, PartitionSpec("core"))
        self.dev_in = [jax.device_put(a, sh) for a in concat_in] + \
                      [jax.device_put(a, sh) for a in concat_zeros]
        for a in self.dev_in:
            a.block_until_ready()

    def run(self):
        outs = self.fn(*self.dev_in)
        for o in outs:
            o.block_until_ready()
        return outs

    def results(self):
        outs = self.run()
        res = [dict() for _ in range(self.n_cores)]
        for i, name in enumerate(self.out_names):
            arr = np.asarray(outs[i]).reshape(self.n_cores, *self.out_avals[i].shape)
            for c in range(self.n_cores):
                res[c][name] = arr[c]
        return res


_EXEC_CACHE = {}


def _get_exec(in_maps, S0, S1):
    key = (S0, S1, REPS, SIMNC, GRP, FP8, NQ, CHUNK)
    if key not in _EXEC_CACHE:
        _EXEC_CACHE[key] = _Exec(_get_nc(S0, S1), in_maps)
    return _EXEC_CACHE[key]


def _run(in_maps, S0, S1):
    nc = _get_nc(S0, S1)
    r = bass_utils.run_bass_kernel_spmd(nc, in_maps, core_ids=list(range(NCORES)), trace=False)
    return r


def kernel(**inputs):
    in_maps, S0, S1 = _prep(inputs)
    r = _run(in_maps, S0, S1)
    out = np.concatenate([r.results[c]["outv"].reshape(-1) for c in range(NCORES)])
    fcb = np.asarray(inputs["fcb"], np.float32).reshape(-1)
    out = (out + fcb[0]).astype(np.float32)[:, None]
    # numerically stable sigmoid in fp32
    sig = np.empty_like(out)
    pos = out >= 0
    sig[pos] = 1.0 / (1.0 + np.exp(-out[pos], dtype=np.float32))
    ex = np.exp(out[~pos], dtype=np.float32)
    sig[~pos] = ex / (1.0 + ex)
    return out, sig
